# revision 63
# baseline (speedup 1.0000x reference)
"""DiT block (Linformer attention + adaLN + MLP) on 8 TRN2 NeuronCores.

Sharding: data-parallel over batch (B=8 -> one batch element per core).

Layout per core (S=2048 tokens, D=1024 features):
 - adaLN: normalize-only in natural [S_p, D_f] layout (bn_stats over free
   dim); the conditioned scale/offset is folded into the PE-transpose
   evacuation (tensor_scalar mult+add with per-feature columns), so the
   scaled x1 is only ever materialized transposed.  The natural tiles stay
   un-scaled; their consumer (P_EF) gets the scale on the psum evacuation
   and the offset folded into the k/v bias tiles via wk^T@o1 / o1@wv rank-1s.
 - Linformer K/V: k_projT = wk^T @ (s1*(xn^T @ Ew)) + bias; v likewise from
   Fw.  Softmax denominators ride a fused ones-column on v_proj (65-row
   attn@V); normalization = DVE reciprocal (via an SBUF copy of the PSUM
   row), a PE ones-matmul row broadcast, and one DVE mult reading attn@V
   straight from PSUM.  The normalize is deferred one head-pair so the PE
   broadcast never head-of-line-blocks the next pair's attn@V; wo runs
   fused at the end of each group's attention.  attn-out reuses qT in place.
 - MLP: m1w is converted once to f16 (resident in SBUF during the MLP),
   m2w to fp8e4m3 scaled by 16 and run with DoubleRow matmuls against
   fp8 gelu outputs scaled by 8 (1/128 folded into the epilogue).
 - All DMAs are batched via rearranged access patterns; dependent small
   DMAs ride the scalar queue so they never stall the sync queue's
   weight streams.
"""
import contextlib

import numpy as np

import concourse.bass as bass
import concourse.mybir as mybir
import concourse.tile as tile
from concourse import bacc
from concourse.bass import ds, ts
from concourse.bass_utils import run_bass_kernel_spmd
from concourse.masks import make_identity

f32 = mybir.dt.float32
f32r = mybir.dt.float32r
f16 = mybir.dt.float16
f8 = mybir.dt.float8e4
AF = mybir.ActivationFunctionType
OP = mybir.AluOpType
PM = mybir.MatmulPerfMode
W2SC = 16.0  # m2w is stored *W2SC in fp8; folded back out in the epilogue

B, S, D, H, K, MLP, ZD = 8, 2048, 1024, 16, 256, 4096, 1024
DH = D // H      # 64
P = 128
SC = S // P      # 16 token chunks of 128
DC = D // P      # 8 feature chunks of 128
NG = 4           # token groups of 512
GS = 512
MC = MLP // P    # 32
KC = K // P      # 2
EPS = 1e-6

W2D = [("wq", D, D), ("wk", D, D), ("wv", D, D), ("wo", D, D),
       ("Ew", S, K), ("Fw", S, K),
       ("h1w", ZD, D), ("g1w", D, D), ("be1w", D, D),
       ("h2w", ZD, D), ("g2w", D, D), ("be2w", D, D),
       ("m1w", D, MLP), ("m2w", MLP, D)]
W1D = [("bq", D), ("bk", D), ("bv", D), ("bo", D), ("Eb", K), ("Fb", K),
       ("h1b", D), ("g1b", D), ("be1b", D), ("h2b", D), ("g2b", D), ("be2b", D),
       ("m1b", MLP), ("m2b", D)]

_cache = {}


def build():
    if "nc" in _cache:
        return _cache["nc"]
    nc = bacc.Bacc("TRN2", target_bir_lowering=False, debug=False, num_devices=8)
    ap = {}
    ap["x"] = nc.dram_tensor("x", [S, D], f32, kind="ExternalInput").ap()
    ap["z"] = nc.dram_tensor("z", [1, ZD], f32, kind="ExternalInput").ap()
    for nm, a, b in W2D:
        ap[nm] = nc.dram_tensor(nm, [a, b], f32, kind="ExternalInput").ap()
    for nm, a in W1D:
        ap[nm] = nc.dram_tensor(nm, [a], f32, kind="ExternalInput").ap()
    out = nc.dram_tensor("out", [S, D], f32, kind="ExternalOutput").ap()
    with tile.TileContext(nc, trace_sim=False) as tc:
        _emit(nc, tc, ap, out)
    nc.compile()
    _cache["nc"] = nc
    return nc


def _dram_pmajor(ap2d, r0, nrows, c0=0, ncols=None):
    """[nrows, ncols] DRAM slice -> [128, nrows//128, ncols] partition-major."""
    if ncols is None:
        ncols = ap2d.shape[1]
    sl = ap2d[ds(r0, nrows), ds(c0, ncols)]
    return sl.rearrange("(a p) d -> p a d", p=P)


def _emit(nc, tc, ap, out):
    ctx = contextlib.ExitStack()
    with ctx:
        # ---------- whole-kernel pools ----------
        const = ctx.enter_context(tc.tile_pool(name="const", bufs=1))
        rows = ctx.enter_context(tc.tile_pool(name="rows", bufs=1))
        cols = ctx.enter_context(tc.tile_pool(name="cols", bufs=1))
        avec = ctx.enter_context(tc.tile_pool(name="avec", bufs=1))
        dram = ctx.enter_context(tc.tile_pool(name="dram", bufs=1, space="DRAM"))

        attn_sc = dram.tile([S, D], f16, tag="attn_sc", name="attn_sc")
        m1h = dram.tile([D, MLP], f16, tag="m1h", name="m1h")
        m2h = dram.tile([MLP, D], f8, tag="m2h", name="m2h")
        vrow_sc = dram.tile([8, D], f32, tag="vrow_sc", name="vrow_sc")

        ident_f = const.tile([P, P], f32, tag="ident_f", name="ident_f")
        make_identity(nc, ident_f)
        ident_h = const.tile([P, P], f16, tag="ident_h", name="ident_h")
        nc.vector.tensor_copy(ident_h[:], ident_f[:])
        eps_t = const.tile([P, 1], f32, tag="eps", name="eps")
        nc.vector.memset(eps_t[:], EPS)
        ones_f = const.tile([P, 1], f32, tag="ones_f", name="ones_f")
        nc.vector.memset(ones_f[:], 1.0)
        ones1_f = const.tile([1, P], f32, tag="ones1_f", name="ones1_f")
        nc.vector.memset(ones1_f[:], 1.0)
        ones1_h = const.tile([1, P], f16, tag="ones1_h", name="ones1_h")
        nc.vector.tensor_copy(ones1_h[:], ones1_f[:])
        onescol_h = const.tile([P, 1], f16, tag="onescol_h", name="onescol_h")
        nc.vector.tensor_copy(onescol_h[:], ones_f[:])
        ones1_r = const.tile([1, P], f32r, tag="ones1_r", name="ones1_r")
        nc.vector.tensor_copy(ones1_r[:], ones1_f[:])

        def col_load(name, n):
            """1-D DRAM vector [n*128] -> sbuf [128, n] (partition-major)."""
            t = cols.tile([P, n], f32, tag=f"cols_{name}", name=f"cols_{name}")
            nc.sync.dma_start(t[:], ap[name].rearrange("(a p) -> p a", p=P))
            return t

        def bcast_rows(tag, row_f, n, psp, pool, rpool=None):
            """[1, n] fp32 row -> [128, n] fp32 tile via ones-matmul."""
            row_r = (rpool or pool).tile([1, n], f16, tag=f"rr_{tag}",
                                         name=f"rr_{tag}")
            nc.vector.tensor_copy(row_r[:], row_f[0:1, 0:n])
            t = pool.tile([P, n], f32, tag=f"bc_{tag}", name=f"bc_{tag}")
            for h in range(0, n, GS):
                w = min(GS, n - h)
                pt = psp.tile([P, GS], f32, tag="bc_ps", name="bc_ps")
                nc.tensor.matmul(pt[:, 0:w], ones1_h[:], row_r[0:1, h:h + w],
                                 start=True, stop=True)
                nc.scalar.copy(t[:, h:h + w], pt[:, 0:w])
            return t

        bq_c = col_load("bq", DC)
        bk_c = col_load("bk", DC)
        Fb_c = col_load("Fb", KC)
        m1b_c = col_load("m1b", MC)

        # manual pool stacks (LIFO per SBUF side)
        s_qT = contextlib.ExitStack()    # left: qT tiles, reused as aoT (-> C3)
        s_wo = contextlib.ExitStack()    # left: wo_r f16 (-> C3)
        s_x1n = contextlib.ExitStack()   # left: natural xn tiles (-> B2)
        s_wq = contextlib.ExitStack()    # left: wq f16 (-> B-mm end)
        s_kv = contextlib.ExitStack()    # left: kpT/vpe (KV -> C2)
        s_vw = contextlib.ExitStack()    # right: phase-A streams (-> A2 end)
        s_pef = contextlib.ExitStack()   # right: pefE/F (B2 -> KV)
        s_ef = contextlib.ExitStack()    # right: EF f16 resident (B2 only)
        s_w1 = contextlib.ExitStack()    # right: resident m1w f16 (C2 -> end)

        qT_p = s_qT.enter_context(tc.tile_pool(name="qT", bufs=1))
        qT = [[qT_p.tile([P, GS], f16, tag=f"qT_{j}_{g}", name=f"qT_{j}_{g}")
               for g in range(NG)] for j in range(DC)]
        aoT = qT  # in-place reuse: attn-out overwrites q after last read
        wo_sb = s_wo.enter_context(tc.tile_pool(name="wo_sb", bufs=1))
        wo_r = wo_sb.tile([P, DC, D], f16, tag="wo_r", name="wo_r")
        x1n_p = s_x1n.enter_context(tc.tile_pool(name="x1nat", bufs=1))
        x1n = [x1n_p.tile([P, D], f16, tag=f"nat{i}", name=f"nat{i}")
               for i in range(SC)]

        # ======== phase A helpers (vec matvec chains, f32r streams) ========
        vw = s_vw.enter_context(tc.tile_pool(name="vw", bufs=3, side="right"))
        arow = s_vw.enter_context(tc.tile_pool(name="arow", bufs=1))
        vps = s_vw.enter_context(tc.tile_pool(name="vec_ps", bufs=2,
                                              space="PSUM"))
        s_ln = contextlib.ExitStack()
        ln_sb = s_ln.enter_context(tc.tile_pool(name="ln1_sb", bufs=2))

        def emit_stats(g):
            """x load + adaLN1 stats/normalize for one token group."""
            xg = ln_sb.tile([P, 4, D], f32, tag="ln_in", name="ln_in", bufs=1)
            nc.sync.dma_start(xg[:], _dram_pmajor(ap["x"], GS * g, GS))
            mvs = []
            sd4 = ln_sb.tile([P, 4], f32, tag="ln_sd4", name="ln_sd4")
            for ii in range(4):
                st = ln_sb.tile([P, 2, 6], f32, tag="ln_st", name="ln_st")
                nc.vector.bn_stats(st[:, 0, :], xg[:, ii, 0:GS])
                nc.vector.bn_stats(st[:, 1, :], xg[:, ii, GS:D])
                mv = ln_sb.tile([P, 2], f32, tag=f"ln_mv{ii}",
                                name=f"ln_mv{ii}", bufs=2)
                nc.vector.bn_aggr(mv[:], st[:])
                nc.scalar.activation(sd4[:, ii:ii + 1], mv[:, 1:2],
                                     AF.Sqrt, bias=eps_t[:])
                mvs.append(mv)
            rstd4 = ln_sb.tile([P, 4], f32, tag="ln_rstd4", name="ln_rstd4")
            nc.vector.reciprocal_approx_fast(rstd4[:], sd4[:])
            for ii in range(4):
                i = 4 * g + ii
                rstd = rstd4[:, ii:ii + 1]
                nmr = ln_sb.tile([P, 1], f32, tag="ln_nmr", name="ln_nmr")
                nc.vector.tensor_scalar(nmr[:], mvs[ii][:, 0:1],
                                        rstd, -1.0, OP.mult, OP.mult)
                nc.scalar.activation(x1n[i][:], xg[:, ii, :],
                                     AF.Identity, bias=nmr[:], scale=rstd)

        emit_stats(0)
        emit_stats(1)

        zc_r = avec.tile([P, DC], f32r, tag="zc_r", name="zc_r")
        nc.sync.dma_start(zc_r[:],
                          ap["z"].rearrange("o (a p) -> p (o a)",
                                            p=P).bitcast(f32r))

        def vec_layer(wname, bname, lhs_cols, act, out_row):
            """out_row[1, D] = act(lhs^T @ w + bias); w streamed as f32r."""
            bias_row = arow.tile([1, D], f32, tag="vbias", name=f"b_{bname}",
                                 bufs=2)
            nc.sync.dma_start(bias_row[:], ap[bname][0:D])
            pts = [vps.tile([1, GS], f32, tag=f"vps{h}", name=f"vps{h}",
                            bufs=1) for h in range(2)]
            for h in range(2):
                wt = vw.tile([P, DC, GS], f32r, tag="vw", name=f"vw_{wname}{h}")
                nc.sync.dma_start(
                    wt[:], _dram_pmajor(ap[wname], 0, D,
                                        GS * h, GS).bitcast(f32r))
                for j in range(DC):
                    nc.tensor.matmul(pts[h][:], lhs_cols[:, j:j + 1],
                                     wt[:, j, :],
                                     start=(j == 0), stop=(j == DC - 1))
            for h in range(2):
                pre = arow.tile([1, GS], f32, tag=f"vpre{h}", name=f"vpre{h}")
                nc.vector.tensor_add(pre[:], pts[h][:],
                                     bias_row[0:1, ds(GS * h, GS)])
                if act is None:
                    nc.vector.tensor_copy(out_row[0:1, ds(GS * h, GS)], pre[:])
                else:
                    nc.scalar.activation(out_row[0:1, ds(GS * h, GS)],
                                         pre[:], act)

        def row_to_cols(tag, slot, row_f, dtype=f32):
            """[1, D] sbuf row -> [128, DC] cols via DRAM roundtrip (on the
            scalar DMA queue -- a dependent DMA at the head of the sync
            queue would stall every later weight load)."""
            nc.scalar.dma_start(vrow_sc[slot:slot + 1, :], row_f[0:1, :])
            cf = avec.tile([P, DC], dtype, tag=f"c_{tag}", name=f"c_{tag}")
            src = vrow_sc[slot:slot + 1, :].rearrange(
                "o (a p) -> p (o a)", p=P)
            if dtype == f32r:
                src = src.bitcast(f32r)
            nc.scalar.dma_start(cf[:], src)
            return cf

        def vrow(nm):
            return arow.tile([1, D], f32, tag="vout", name=nm, bufs=3)

        # ========= phase A1: h1 -> s1/o1 (leads the DMA queues) =========
        h1_row = vrow("h1")
        vec_layer("h1w", "h1b", zc_r[:], AF.Silu, h1_row)
        h1_c = row_to_cols("h1", 0, h1_row, f32r)
        sc1_row = vrow("sc1")
        vec_layer("g1w", "g1b", h1_c[:], None, sc1_row)
        of1_row = vrow("of1")
        vec_layer("be1w", "be1b", h1_c[:], None, of1_row)
        s1_c = row_to_cols("s1", 1, sc1_row)
        o1_c = row_to_cols("o1", 2, of1_row)
        o1c_h = avec.tile([P, DC], f16, tag="o1c_h", name="o1c_h")
        nc.vector.tensor_copy(o1c_h[:], o1_c[:])

        # ===== phase B0b: remaining stats + wq prefetch ==
        emit_stats(2)
        emit_stats(3)
        s_ln.close()
        wq_sb = s_wq.enter_context(tc.tile_pool(name="wq_sb", bufs=1))
        wq_r = wq_sb.tile([P, DC, D], f16, tag="wq_r", name="wq_r")
        with tc.tile_pool(name="wq_st", bufs=1) as wqst:
            for hf in range(4):
                wqf = wqst.tile([P, 2, D], f32, tag="wq_f", name="wq_f",
                                bufs=2)
                nc.sync.dma_start(wqf[:],
                                  _dram_pmajor(ap["wq"], 2 * P * hf, 2 * P))
                nc.vector.tensor_copy(wq_r[:, ds(2 * hf, 2), :], wqf[:])

        try:
            # ===== phase B: raw transposes + wq-side s1/o1 fold + qT =====
            # x1T holds the *unscaled* xn^T; the adaLN1 scale rides on wq
            # (wq_s = diag(s1) @ wq) and the offset on the q bias column
            # (bqx = bq + wq^T @ o1), so the transpose evacuations have no
            # dependency on the conditioning chain.
            with (
                tc.tile_pool(name="x1Trot", bufs=2) as x1t_p,
                tc.tile_pool(name="bqx_sb", bufs=1) as bqxp,
                tc.tile_pool(name="tp1_ps", bufs=2, space="PSUM") as ln_ps,
                tc.tile_pool(name="q_ps", bufs=3, space="PSUM") as qps,
            ):
                # bqx = bq + wq^T @ o1 (uses un-scaled wq), then scale wq
                pts = [vps.tile([1, GS], f32, tag=f"vps{h}", name=f"vps{h}",
                                bufs=1) for h in range(2)]
                for j in range(DC):
                    for h in range(2):
                        nc.tensor.matmul(pts[h][:], o1c_h[:, j:j + 1],
                                         wq_r[:, j, ds(GS * h, GS)],
                                         start=(j == 0), stop=(j == DC - 1))
                bq1_row = bqxp.tile([1, D], f32, tag="bq1_row", name="bq1_row")
                for h in range(2):
                    nc.vector.tensor_copy(bq1_row[0:1, ds(GS * h, GS)],
                                          pts[h][:])
                nc.scalar.dma_start(vrow_sc[7:8, :], bq1_row[0:1, :])
                bq1_c = bqxp.tile([P, DC], f32, tag="bq1_c", name="bq1_c")
                nc.scalar.dma_start(bq1_c[:],
                                    vrow_sc[7:8, :].rearrange(
                                        "o (a p) -> p (o a)", p=P))
                bqx_c = bqxp.tile([P, DC], f32, tag="bqx_c", name="bqx_c")
                nc.vector.tensor_add(bqx_c[:], bq1_c[:], bq_c[:])
                for j in range(DC):
                    nc.vector.tensor_scalar(wq_r[:, j, :], wq_r[:, j, :],
                                            s1_c[:, j:j + 1], None, OP.mult)

                for g in range(NG):
                    x1T_g = [x1t_p.tile([P, GS], f16, tag=f"x1T_{j}",
                                        name=f"x1T_{j}") for j in range(DC)]
                    for ii in range(4):
                        i = 4 * g + ii
                        for j in range(DC):
                            pt = ln_ps.tile([P, P], f16, tag="tp_ps",
                                            name="tp_ps")
                            nc.tensor.transpose(pt[:], x1n[i][:, ds(P * j, P)],
                                                ident_h[:])
                            if j % 2 == 0:
                                nc.vector.tensor_copy(
                                    x1T_g[j][:, ds(P * ii, P)], pt[:])
                            else:
                                nc.scalar.copy(
                                    x1T_g[j][:, ds(P * ii, P)], pt[:])
                    for jo in range(DC):
                        pt = qps.tile([P, GS], f32, tag="q_ps", name="q_ps")
                        for j in range(DC):
                            nc.tensor.matmul(pt[:],
                                             wq_r[:, j, ds(P * jo, P)],
                                             x1T_g[j][:],
                                             start=(j == 0),
                                             stop=(j == DC - 1))
                        nc.scalar.activation(qT[jo][g][:], pt[:], AF.Identity,
                                             bias=bqx_c[:, jo:jo + 1])
            s_wq.close()

            # ========= phase A2: h2 -> s2/o2 (needed only in phase D) ======
            h2_row = vrow("h2")
            vec_layer("h2w", "h2b", zc_r[:], AF.Silu, h2_row)
            h2_c = row_to_cols("h2", 3, h2_row, f32r)
            sc2_row = vrow("sc2")
            vec_layer("g2w", "g2b", h2_c[:], None, sc2_row)
            of2_row = vrow("of2")
            vec_layer("be2w", "be2b", h2_c[:], None, of2_row)
            s2_c = row_to_cols("s2", 4, sc2_row)
            o2_c = row_to_cols("o2", 5, of2_row)
            s_vw.close()

            # ===== phase B2: P_EF = xn^T @ [Ew|Fw] + colsums ====
            pef_sb = s_pef.enter_context(
                tc.tile_pool(name="pef_sb", bufs=1, side="right"))
            pefE = [pef_sb.tile([P, K], f16, tag=f"pefE{j}", name=f"pefE{j}")
                    for j in range(DC)]
            pefF = [pef_sb.tile([P, K], f16, tag=f"pefF{j}", name=f"pefF{j}")
                    for j in range(DC)]
            ef_sb = s_ef.enter_context(
                tc.tile_pool(name="ef_sb", bufs=1, side="right"))
            EFr = ef_sb.tile([P, SC, 2 * K], f16, tag="EFr", name="EFr")
            cs_row = rows.tile([1, 2 * K], f32, tag="cs", name="cs")
            with (
                tc.tile_pool(name="ef2_sb", bufs=2) as ef2sb,
                tc.tile_pool(name="pef_ps", bufs=1, space="PSUM") as pfps,
                tc.tile_pool(name="cs_ps", bufs=1, space="PSUM") as csps,
            ):
                for half, nm in enumerate(("Ew", "Fw")):
                    eff = ef2sb.tile([P, SC, K], f32, tag="ef_f", name="ef_f")
                    nc.sync.dma_start(eff[:], _dram_pmajor(ap[nm], 0, S))
                    nc.vector.tensor_copy(EFr[:, :, ds(K * half, K)], eff[:])
                pef_ps = [pfps.tile([P, 2 * K], f32, tag=f"pefp{jj}",
                                    name=f"pefp{jj}") for jj in range(4)]
                for wave in range(2):
                    for i in range(SC):
                        for jj in range(4):
                            j = 4 * wave + jj
                            nc.tensor.matmul(pef_ps[jj][:],
                                             x1n[i][:, ds(P * j, P)],
                                             EFr[:, i, :],
                                             start=(i == 0), stop=(i == SC - 1))
                    for jj in range(4):
                        j = 4 * wave + jj
                        nc.vector.tensor_scalar(pefE[j][:],
                                                pef_ps[jj][:, 0:K],
                                                s1_c[:, j:j + 1], None,
                                                OP.mult)
                        nc.vector.tensor_scalar(pefF[j][:],
                                                pef_ps[jj][:, K:2 * K],
                                                s1_c[:, j:j + 1], None,
                                                OP.mult)
                cs_ps = csps.tile([1, 2 * K], f32, tag="cs_ps", name="cs_ps")
                for i in range(SC):
                    nc.tensor.matmul(cs_ps[:], onescol_h[:], EFr[:, i, :],
                                     start=(i == 0), stop=(i == SC - 1))
                nc.vector.tensor_copy(cs_row[:], cs_ps[:])
            s_ef.close()
            s_x1n.close()

            # ===== phase KV: biases, k_projT, v_proj_ext =====
            kv_sb = s_kv.enter_context(tc.tile_pool(name="kv_sb", bufs=1))
            kpT = [kv_sb.tile([P, K], f16, tag=f"kpT{j}", name=f"kpT{j}")
                   for j in range(DC)]
            vpe = [kv_sb.tile([P, 65 * H], f16, tag=f"vpe{c}", name=f"vpe{c}")
                   for c in range(KC)]
            with (
                tc.tile_pool(name="kv_st", bufs=2) as kvst,
                tc.tile_pool(name="kv_wv", bufs=1) as kvwv,
                tc.tile_pool(name="kv_bias", bufs=1) as kvb,
                tc.tile_pool(name="r1_ps", bufs=1, space="PSUM") as r1ps,
                tc.tile_pool(name="bc1_ps", bufs=1, space="PSUM") as bcps,
                tc.tile_pool(name="kv_ps", bufs=2, space="PSUM") as kvps,
            ):
                # wk/wv/wo: streamed in half-chunks + f16 cast
                wcast = {}
                for nm in ("wk", "wv", "wo"):
                    if nm == "wo":
                        wr = wo_r
                    else:
                        wr = kvwv.tile([P, DC, D], f16, tag=f"{nm}_r",
                                       name=f"{nm}_r")
                    for hf in range(2):
                        wf = kvst.tile([P, 4, D], f32, tag="w_f",
                                       name=f"{nm}_f{hf}")
                        nc.sync.dma_start(wf[:],
                                          _dram_pmajor(ap[nm], 4 * P * hf,
                                                       4 * P))
                        if hf == 1:
                            nc.scalar.copy(wr[:, ds(4 * hf, 4), :], wf[:])
                        else:
                            nc.vector.tensor_copy(wr[:, ds(4 * hf, 4), :],
                                                  wf[:])
                    wcast[nm] = wr
                wk_r, wv_r = wcast["wk"], wcast["wv"]

                bv_row = kvb.tile([1, D], f32, tag="bv_row", name="bv_row")
                nc.sync.dma_start(bv_row[:], ap["bv"][0:D])
                Eb_row = kvb.tile([1, K], f32, tag="Eb_row", name="Eb_row")
                nc.sync.dma_start(Eb_row[:], ap["Eb"][0:K])

                # rank-1 offset folds: wk^T@o1 (k bias), o1@wv (v bias);
                # sequential through one psum ring slot (2 banks)
                wk1_row = kvb.tile([1, D], f32, tag="wk1_row", name="wk1_row")
                bvx_row = kvb.tile([1, D], f32, tag="bvx_row", name="bvx_row")
                k1_ps = r1ps.tile([1, D], f32, tag="r1_ps", name="k1_ps")
                for hh in range(2):
                    for j in range(DC):
                        nc.tensor.matmul(k1_ps[0:1, ds(GS * hh, GS)],
                                         o1c_h[:, j:j + 1],
                                         wk_r[:, j, ds(GS * hh, GS)],
                                         start=(j == 0), stop=(j == DC - 1))
                nc.vector.tensor_copy(wk1_row[:], k1_ps[:])
                v1_ps = r1ps.tile([1, D], f32, tag="r1_ps", name="v1_ps")
                for hh in range(2):
                    for j in range(DC):
                        nc.tensor.matmul(v1_ps[0:1, ds(GS * hh, GS)],
                                         o1c_h[:, j:j + 1],
                                         wv_r[:, j, ds(GS * hh, GS)],
                                         start=(j == 0), stop=(j == DC - 1))
                nc.vector.tensor_add(bvx_row[:], v1_ps[:], bv_row[:])

                # wk1 row -> cols (DRAM roundtrip slot 6), add bk.
                # Roundtrips go on the scalar DMA queue: a dependent DMA at
                # the head of the sync queue would stall later weight loads.
                nc.scalar.dma_start(vrow_sc[6:7, :], wk1_row[0:1, :])
                wk1_c = kvb.tile([P, DC], f32, tag="wk1_c", name="wk1_c")
                nc.scalar.dma_start(wk1_c[:],
                                    vrow_sc[6:7, :].rearrange(
                                        "o (a p) -> p (o a)", p=P))
                bkx_c = kvb.tile([P, DC], f32, tag="bkx_c", name="bkx_c")
                nc.vector.tensor_add(bkx_c[:], wk1_c[:], bk_c[:])

                csE_b = bcast_rows("csE", cs_row, K, bcps, kvb, kvst)
                Eb_b = bcast_rows("Eb", Eb_row, K, bcps, kvb, kvst)
                bvx_b = bcast_rows("bvx", bvx_row, D, bcps, kvb, kvst)
                # bo/m2b broadcasts for the later phases (ctx-lifetime pool)
                bo_row = kvb.tile([1, D], f32, tag="bo_row", name="bo_row")
                nc.sync.dma_start(bo_row[:], ap["bo"][0:D])
                bo_b = bcast_rows("bo", bo_row, D, bcps, cols, kvst)
                m2b_row = kvb.tile([1, D], f32, tag="m2b_row", name="m2b_row")
                nc.sync.dma_start(m2b_row[:], ap["m2b"][0:D])
                m2b_b = bcast_rows("m2b", m2b_row, D, bcps, cols, kvst)
                csF_c = kvb.tile([P, KC], f32, tag="csF_c", name="csF_c")
                for c in range(KC):
                    nc.scalar.dma_start(csF_c[:, c:c + 1],
                                        cs_row[0:1, ds(K + P * c, P)])

                kp_bias = []
                for j in range(DC):
                    bt = kvb.tile([P, K], f32, tag=f"kpb{j}", name=f"kpb{j}")
                    nc.vector.scalar_tensor_tensor(bt[:], csE_b[:],
                                                   bkx_c[:, j:j + 1], Eb_b[:],
                                                   OP.mult, OP.add)
                    kp_bias.append(bt)
                vp_bias = []
                for c in range(KC):
                    bt = kvb.tile([P, D], f32, tag=f"vpb{c}", name=f"vpb{c}")
                    nc.vector.tensor_scalar(bt[:], bvx_b[:], csF_c[:, c:c + 1],
                                            Fb_c[:, c:c + 1], OP.mult, OP.add)
                    vp_bias.append(bt)

                for jo in range(DC):
                    pt = kvps.tile([P, K], f32, tag="kp_ps", name="kp_ps")
                    for j in range(DC):
                        nc.tensor.matmul(pt[:], wk_r[:, j, ds(P * jo, P)],
                                         pefE[j][:],
                                         start=(j == 0), stop=(j == DC - 1))
                    nc.vector.tensor_add(kpT[jo][:], pt[:], kp_bias[jo][:])
                # ones columns for the fused softmax denominator
                for c in range(KC):
                    oc = vpe[c][:].rearrange("p (a b) -> p a b", b=65)
                    nc.vector.memset(oc[:, :, 64:65], 1.0)
                for hf in range(2):
                    for c in range(KC):
                        pt = kvps.tile([P, GS], f32, tag="vp_ps", name="vp_ps")
                        for j in range(DC):
                            nc.tensor.matmul(pt[:], pefF[j][:, ds(P * c, P)],
                                             wv_r[:, j, ds(GS * hf, GS)],
                                             start=(j == 0), stop=(j == DC - 1))
                        dst = vpe[c][:, ds(65 * 8 * hf, 65 * 8)].rearrange(
                            "p (a b) -> p a b", b=65)[:, :, 0:64]
                        nc.vector.tensor_add(
                            dst,
                            pt[:].rearrange("p (a b) -> p a b", b=64),
                            vp_bias[c][:, ds(GS * hf, GS)].rearrange(
                                "p (a b) -> p a b", b=64))
            s_pef.close()

            # ===== phase C2+C3: attention, wo, residual (fused per group) ===
            with (
                tc.tile_pool(name="at_sb", bufs=6) as atsb,
                tc.tile_pool(name="rc_sb", bufs=2) as rcsb,
                tc.tile_pool(name="bb_sb", bufs=4) as bbsb,
                tc.tile_pool(name="c3_sb", bufs=2) as c3sb,
                tc.tile_pool(name="wcv", bufs=2) as wcv,
                tc.tile_pool(name="sc_ps", bufs=2, space="PSUM") as scps,
                tc.tile_pool(name="av_ps", bufs=4, space="PSUM") as avps,
                tc.tile_pool(name="bc2_ps", bufs=1, space="PSUM") as bc2ps,
                tc.tile_pool(name="wo_ps", bufs=1, space="PSUM") as wops,
            ):
                # m1w -> f16 and m2w -> scaled fp8 DRAM scratch.  The DMAs
                # drain during the attention phase; the casts run on ACT/DVE
                # interleaved with it.
                for j in range(DC):
                    for h2 in range(4):
                        wf = wcv.tile([P, MLP // 4], f32, tag="m1f",
                                      name="m1f")
                        nc.sync.dma_start(wf[:],
                                          ap["m1w"][ds(P * j, P),
                                                    ds(MLP // 4 * h2, MLP // 4)])
                        wh = wcv.tile([P, MLP // 4], f16, tag="m1hh",
                                      name="m1hh")
                        if h2 % 2 == 0:
                            nc.scalar.copy(wh[:], wf[:])
                        else:
                            nc.vector.tensor_copy(wh[:], wf[:])
                        nc.sync.dma_start(
                            m1h[ds(P * j, P), ds(MLP // 4 * h2, MLP // 4)],
                            wh[:])
                for jj in range(2 * DC):
                    wf = wcv.tile([P, 2, D], f32, tag="m2f", name="m2f")
                    nc.sync.dma_start(wf[:],
                                      _dram_pmajor(ap["m2w"], 2 * P * jj, 2 * P))
                    wh = wcv.tile([P, 2, D], f8, tag="m2hh", name="m2hh")
                    nc.vector.tensor_scalar(wh[:], wf[:], W2SC, None, OP.mult)
                    nc.sync.dma_start(
                        m2h[ds(2 * P * jj, 2 * P), :].rearrange(
                            "(a p) d -> p a d", p=P), wh[:])
                def emit_scores_pair(g, p):
                    """exp(scores) for heads 2p, 2p+1 (PE row groups 0/64)."""
                    exps = [[None, None], [None, None]]
                    for c in range(KC):
                        spts = []
                        for e in range(2):
                            r0 = 64 * e
                            spt = scps.tile([P, GS], f32, tag="sc", name="sc")
                            nc.tensor.matmul(spt[:],
                                             kpT[p][r0:r0 + 64, ds(P * c, P)],
                                             qT[p][g][r0:r0 + 64, :],
                                             start=True, stop=True)
                            spts.append(spt)
                        for e in range(2):
                            et = atsb.tile([P, GS], f16, tag="exp", name="exp")
                            nc.scalar.activation(et[:], spts[e][:], AF.Exp,
                                                 scale=0.125)
                            exps[e][c] = et
                    return exps

                for g in range(NG):
                    xg = c3sb.tile([P, 4, D], f32, tag="res_x", name="res_x",
                                   bufs=1)
                    nc.sync.dma_start(xg[:], _dram_pmajor(ap["x"], GS * g, GS))
                    atg = c3sb.tile([P, 4, D], f16, tag="atg", name="atg")
                    def norm_flush(apts, rcph, p):
                        # softmax normalize for pair p (deferred one pair so
                        # the PE bcast never blocks the next pair's attnV)
                        bpt = bc2ps.tile([P, GS], f32, tag="bc2", name="bc2")
                        for e in range(2):
                            nc.tensor.matmul(
                                bpt[ds(64 * e, 64), :], ones1_h[0:1, 0:64],
                                rcph[0:1, ds(GS * e, GS)],
                                start=True, stop=True)
                        bsb = bbsb.tile([P, GS], f32, tag="bb", name="bb")
                        nc.vector.tensor_copy(bsb[:], bpt[:])
                        for e in range(2):
                            r0 = 64 * e
                            nc.vector.tensor_mul(aoT[p][g][r0:r0 + 64, :],
                                                 apts[e][0:64, :],
                                                 bsb[r0:r0 + 64, :])

                    exps = emit_scores_pair(g, 0)
                    pend = None
                    for p in range(8):
                        nxt = emit_scores_pair(g, p + 1) if p < 7 else None
                        apts = []
                        den2 = rcsb.tile([1, 2 * GS], f32, tag="den2",
                                         name="den2")
                        for e in range(2):
                            h = 2 * p + e
                            apt = avps.tile([65, GS], f32, tag="av", name="av")
                            for c in range(KC):
                                nc.tensor.matmul(apt[:],
                                                 vpe[c][:, ds(65 * h, 65)],
                                                 exps[e][c][:],
                                                 start=(c == 0),
                                                 stop=(c == KC - 1))
                            if e == 0:
                                nc.scalar.copy(den2[0:1, ds(GS * e, GS)],
                                               apt[64:65, :])
                            else:
                                nc.vector.tensor_copy(den2[0:1, ds(GS * e, GS)],
                                                      apt[64:65, :])
                            apts.append(apt)
                        rcp = rcsb.tile([1, 2 * GS], f32, tag="rcp",
                                        name="rcp")
                        nc.vector.reciprocal_approx_fast(rcp[:], den2[:])
                        rcph = rcsb.tile([1, 2 * GS], f16, tag="rcph",
                                         name="rcph")
                        nc.vector.tensor_copy(rcph[:], rcp[:])
                        if pend is not None:
                            norm_flush(*pend)
                        pend = (apts, rcph, p)
                        exps = nxt
                    norm_flush(*pend)
                    # wo + bo + x residual for this group -> attn_sc
                    for ii in range(4):
                        xb = c3sb.tile([P, D], f16, tag="xb", name="xb",
                                       bufs=4)
                        nc.vector.tensor_add(xb[:], xg[:, ii, :], bo_b[:])
                        for hf in range(2):
                            pt = wops.tile([P, GS], f32, tag="wo_ps",
                                           name="wo_ps")
                            for j in range(DC):
                                nc.tensor.matmul(pt[:],
                                                 aoT[j][g][:, ds(P * ii, P)],
                                                 wo_r[:, j, ds(GS * hf, GS)],
                                                 start=(j == 0),
                                                 stop=(j == DC - 1))
                            nc.vector.scalar_tensor_tensor(
                                atg[:, ii, ds(GS * hf, GS)], pt[:], 1.0,
                                xb[:, ds(GS * hf, GS)], OP.mult, OP.add)
                    nc.sync.dma_start(
                        attn_sc[ds(GS * g, GS), :].rearrange(
                            "(a p) d -> p a d", p=P), atg[:])
            s_kv.close()
            s_x1n.close()
            s_wo.close()
            s_qT.close()

            # ===== phase D+E: adaLN2 -> MLP, fused per token-group =====
            if True:
                # resident m1w f16, loaded once from the converted scratch
                w1_sb = s_w1.enter_context(
                    tc.tile_pool(name="w1_sb", bufs=1, side="right"))
                w1r = [w1_sb.tile([P, MLP], f16, tag=f"w1r{j}", name=f"w1r{j}")
                       for j in range(DC)]
                for j in range(DC):
                    nc.scalar.dma_start(w1r[j][:], m1h[ds(P * j, P), :])
                with (
                    tc.tile_pool(name="ln2_sb", bufs=2) as ln2_sb,
                    tc.tile_pool(name="x2Trot", bufs=2) as x2t_p,
                    tc.tile_pool(name="hmid", bufs=1) as hmid_p,
                    tc.tile_pool(name="m2st", bufs=2) as m2st,
                    tc.tile_pool(name="e_sb", bufs=1) as esb,
                    tc.tile_pool(name="tp2_ps", bufs=2, space="PSUM") as ln2_ps,
                    tc.tile_pool(name="m1_ps", bufs=2, space="PSUM") as m1ps,
                    tc.tile_pool(name="m2_ps", bufs=1, space="PSUM") as m2ps,
                ):
                    for g in range(NG):
                        ag = ln2_sb.tile([P, 4, D], f16, tag="ln2_in",
                                         name="ln2_in")
                        nc.sync.dma_start(
                            ag[:], attn_sc[ds(GS * g, GS), :].rearrange(
                                "(a p) d -> p a d", p=P))
                        resb = []
                        for ii in range(4):
                            rb = ln2_sb.tile([P, D], f16, tag=f"resb{ii}",
                                             name=f"resb{ii}")
                            nc.vector.tensor_add(rb[:], ag[:, ii, :], m2b_b[:])
                            resb.append(rb)
                        x2T_g = [x2t_p.tile([P, GS], f16, tag=f"x2T_{j}",
                                            name=f"x2T_{j}") for j in range(DC)]
                        mvs = []
                        sd4 = ln2_sb.tile([P, 4], f32, tag="ln2_sd4",
                                          name="ln2_sd4")
                        for ii in range(4):
                            st = ln2_sb.tile([P, 2, 6], f32, tag="ln2_st",
                                             name="ln2_st")
                            nc.vector.bn_stats(st[:, 0, :], ag[:, ii, 0:GS])
                            nc.vector.bn_stats(st[:, 1, :], ag[:, ii, GS:D])
                            mv = ln2_sb.tile([P, 2], f32, tag=f"ln2_mv{ii}",
                                             name=f"ln2_mv{ii}", bufs=2)
                            nc.vector.bn_aggr(mv[:], st[:])
                            nc.scalar.activation(sd4[:, ii:ii + 1], mv[:, 1:2],
                                                 AF.Sqrt, bias=eps_t[:])
                            mvs.append(mv)
                        rstd4 = ln2_sb.tile([P, 4], f32, tag="ln2_rstd4",
                                            name="ln2_rstd4")
                        nc.vector.reciprocal_approx_fast(rstd4[:], sd4[:])
                        for ii in range(4):
                            rstd = rstd4[:, ii:ii + 1]
                            nmr = ln2_sb.tile([P, 1], f32, tag="ln2_nmr",
                                              name="ln2_nmr")
                            nc.vector.tensor_scalar(nmr[:], mvs[ii][:, 0:1],
                                                    rstd, -1.0,
                                                    OP.mult, OP.mult)
                            xn2 = ln2_sb.tile([P, D], f16, tag="ln2_xn",
                                              name="ln2_xn")
                            nc.scalar.activation(xn2[:], ag[:, ii, :],
                                                 AF.Identity, bias=nmr[:],
                                                 scale=rstd)
                            for j in range(DC):
                                pt = ln2_ps.tile([P, P], f16, tag="tp2_ps",
                                                 name="tp2_ps")
                                nc.tensor.transpose(pt[:], xn2[:, ds(P * j, P)],
                                                    ident_h[:])
                                if j % 2 == 0:
                                    nc.vector.tensor_scalar(
                                        x2T_g[j][:, ds(P * ii, P)], pt[:],
                                        s2_c[:, j:j + 1], o2_c[:, j:j + 1],
                                        OP.mult, OP.add)
                                else:
                                    nc.scalar.activation(
                                        x2T_g[j][:, ds(P * ii, P)], pt[:],
                                        AF.Identity, bias=o2_c[:, j:j + 1],
                                        scale=s2_c[:, j:j + 1])
                        # hm2[c] holds gelu(m1) for the m-tile pair (2c, 2c+1)
                        # as fp8 in DoubleRow lhsT layout [Ki, Ko=2, tok]
                        hm2 = [hmid_p.tile([P, 2, GS], f8, tag=f"hm2_{c}",
                                           name=f"hm2_{c}")
                               for c in range(MC // 2)]
                        outg = esb.tile([P, 4, D], f32, tag="e_out",
                                        name="e_out")
                        for half in range(2):
                            m2p = [m2ps.tile([P, GS], f32, tag=f"m2p{ss}",
                                             name=f"m2p{ss}") for ss in range(4)]
                            w2blk = None
                            for c in range(MC // 2):
                                if half == 0:
                                    for o in range(2):
                                        m = 2 * c + o
                                        pt = m1ps.tile([P, GS], f32, tag="m1p",
                                                       name="m1p")
                                        for j in range(DC):
                                            nc.tensor.matmul(
                                                pt[:], w1r[j][:, ds(P * m, P)],
                                                x2T_g[j][:],
                                                start=(j == 0),
                                                stop=(j == DC - 1))
                                        hmt = ln2_sb.tile([P, GS], f16,
                                                          tag="hmt",
                                                          name="hmt", bufs=3)
                                        nc.scalar.activation(
                                            hmt[:], pt[:], AF.Gelu,
                                            bias=m1b_c[:, m:m + 1])
                                        nc.vector.tensor_scalar(
                                            hm2[c][:, o, :], hmt[:], 8.0,
                                            None, OP.mult)
                                if c % 2 == 0:
                                    # two pairs of m2w rows, DoubleRow layout
                                    w2blk = m2st.tile([P, 2, 2, GS], f8,
                                                      tag="m2r", name="m2r")
                                    src = m2h[ds(2 * P * c, 4 * P),
                                              ds(GS * half, GS)].rearrange(
                                        "(a o p) d -> p a o d", p=P, o=2)
                                    nc.sync.dma_start(w2blk[:], src)
                                for ss in range(4):
                                    nc.tensor.matmul(
                                        m2p[ss][:],
                                        hm2[c][:, :, ds(P * ss, P)],
                                        w2blk[:, c % 2, :, :],
                                        start=(c == 0), stop=(c == MC // 2 - 1),
                                        perf_mode=PM.DoubleRow)
                            for ss in range(4):
                                nc.vector.scalar_tensor_tensor(
                                    outg[:, ss, ds(GS * half, GS)],
                                    m2p[ss][:], 1.0 / (8.0 * W2SC),
                                    resb[ss][:, ds(GS * half, GS)],
                                    OP.mult, OP.add)
                        nc.sync.dma_start(
                            out[ds(GS * g, GS), :].rearrange(
                                "(a p) d -> p a d", p=P), outg[:])
        finally:
            for s in (s_kv, s_wq, s_x1n, s_wo, s_qT, s_w1, s_ef, s_pef, s_vw):
                s.close()


def kernel(**inputs):
    nc = build()
    x = np.ascontiguousarray(inputs["x"], dtype=np.float32)
    z = np.ascontiguousarray(inputs["z"], dtype=np.float32)
    base = {}
    for nm, _, _ in W2D:
        base[nm] = np.ascontiguousarray(inputs[nm], dtype=np.float32)
    for nm, _ in W1D:
        base[nm] = np.ascontiguousarray(inputs[nm], dtype=np.float32)
    in_maps = []
    for c in range(B):
        m = dict(base)
        m["x"] = x[c]
        m["z"] = z[c:c + 1]
        in_maps.append(m)
    res = run_bass_kernel_spmd(nc, in_maps, list(range(B)))
    _cache["last"] = res
    return np.stack([res.results[c]["out"] for c in range(B)], axis=0)
